# revision 1
# baseline (speedup 1.0000x reference)
"""AdaGAE forward on 8 Trainium2 NeuronCores.

reference:
    h   = relu(spmm(X @ W1))        # spmm = COO Laplacian x dense
    emb = spmm(h @ W2)
    out = softmax(-pairwise_sqdist(emb), axis=1) + 1e-10   # [8192, 8192]

Sharding: nodes row-sharded, 1024 rows/core. Each spmm gathers rows of the
all-gathered table with dma_gather and reduces per-128-edge chunks with a
one-hot matmul G.T @ S that also transposes (h and emb are kept feature-major).
All matmuls run in bf16 (f32 PSUM accumulate); sq is computed from the SAME
quantized embedding so the softmax stays consistent (validated: full-bf16
pipeline L2 rel err ~1e-5 vs 2e-2 gate). The NxN phase computes
z = 2*emb_i.emb_j - sq_j via one K=66 bf16 matmul per [128,512] block
(-sq_j folded in as two extra contraction rows, bf16 hi+lo), then
exp(z - sq_i) on ScalarE (row max of -dist is exactly 0, bias=-sq_i in f32)
with the row-sum fused via accum_out, one VectorE normalize (*1/s + 1e-10)
pass, DMA out.
"""

import os
import sys

if "/opt/trn_rl_repo" not in sys.path:
    sys.path.insert(0, "/opt/trn_rl_repo")

import ml_dtypes
import numpy as np

import concourse.bacc as bacc
import concourse.bass as bass
import concourse.mybir as mybir
from concourse.tile import TileContext
from concourse.bass_utils import run_bass_kernel_spmd

NC = 8          # cores
N = 8192        # nodes
R = N // NC     # rows per core
P = 128
DIN = 1024
DMID = 256
DEMB = 64
KC = DIN // P   # k-chunks for X @ W1
NBLK = R // P   # 128-row blocks per core
Y2W = 128       # Y2 table row padded to 128 bf16 cols (gather needs 256B rows)

F32 = mybir.dt.float32
BF16 = mybir.dt.bfloat16
I16 = mybir.dt.int16
BFNP = ml_dtypes.bfloat16

LAST_RESULTS = None  # BassKernelResults of the most recent run (for test.py)

_GRAPH_CACHE = {}


def _build(b_ch: int):
    """Build the per-core Bass graph. b_ch = gather chunks per 128-row block."""
    kphases = int(os.environ.get("KPHASES", "9"))
    c1 = NBLK * b_ch          # total chunks per core
    ne = c1 * P               # padded edges per core
    nc = bacc.Bacc(None, target_bir_lowering=False, debug=False, num_devices=NC,
                   num_swdge_queues=4)

    xT = nc.declare_dram_parameter("xT", [DIN, R], BF16, isOutput=False)
    w1 = nc.declare_dram_parameter("w1", [DIN, DMID], BF16, isOutput=False)
    w2 = nc.declare_dram_parameter("w2", [DMID, DEMB], BF16, isOutput=False)
    idx = nc.declare_dram_parameter("idx", [128, ne // 16], I16, isOutput=False)
    smat = nc.declare_dram_parameter("smat", [128, c1 * P], BF16, isOutput=False)
    out = nc.declare_dram_parameter("out", [R, N], F32, isOutput=True)

    ag1_in = nc.dram_tensor("ag1_in", [R, DMID], BF16)
    ag1_out = nc.dram_tensor("ag1_out", [N, DMID], BF16, addr_space="Shared")
    ag2_in = nc.dram_tensor("ag2_in", [R, Y2W], BF16)
    ag2_out = nc.dram_tensor("ag2_out", [N, Y2W], BF16, addr_space="Shared")
    ag3_in = nc.dram_tensor("ag3_in", [DEMB + 2, R], BF16)
    ag3_out = nc.dram_tensor("ag3_out", [NC * (DEMB + 2), R], BF16,
                             addr_space="Shared")

    rg = [list(range(NC))]

    with (
        TileContext(nc) as tc,
        tc.tile_pool(name="sb", bufs=1) as sb,
    ):
        ps = tc.alloc_tile_pool(name="ps", bufs=1, space="PSUM")

        def bank1(name):
            return ps.tile([P, 512], F32, tag="bank1", bufs=6, name=name)

        # ---- persistent gather bookkeeping ---------------------------------
        idx_t = sb.tile([128, ne // 16], I16, tag="idx", bufs=1)
        nc.sync.dma_start(out=idx_t[:], in_=idx[:, :])
        w2a = sb.tile([P, DEMB], BF16, tag="w2a", bufs=1)
        nc.sync.dma_start(out=w2a[:], in_=w2[0:P, :])
        w2b = sb.tile([P, DEMB], BF16, tag="w2b", bufs=1)
        nc.sync.dma_start(out=w2b[:], in_=w2[P:2 * P, :])

        # multi-hot (val-folded) selection matrices, shared by both spmms
        s_all = sb.tile([P, c1 * P], BF16, tag="sall", bufs=1)
        nc.sync.dma_start(out=s_all[:], in_=smat[:, :])

        # ---- phase A: Y1 = X @ W1 (row-sharded) -> ag1_in ------------------
        with tc.tile_pool(name="pa", bufs=1) as pa:
            xts, w1s = [], []
            for k in range(KC):
                xt_t = pa.tile([P, R], BF16, tag="xt", bufs=KC, name=f"xt_{k}")
                nc.sync.dma_start(out=xt_t[:], in_=xT[k * P:(k + 1) * P, :])
                xts.append(xt_t)
                w1_t = pa.tile([P, DMID], BF16, tag="w1", bufs=KC, name=f"w1_{k}")
                nc.sync.dma_start(out=w1_t[:], in_=w1[k * P:(k + 1) * P, :])
                w1s.append(w1_t)
            for m in range(NBLK):
                py1 = bank1(f"py1_{m}")
                for k in range(KC):
                    nc.tensor.matmul(
                        out=py1[:, 0:DMID],
                        lhsT=xts[k][:, m * P:(m + 1) * P],
                        rhs=w1s[k][:],
                        start=(k == 0),
                        stop=(k == KC - 1),
                    )
                y1 = pa.tile([P, DMID], BF16, tag="y1", bufs=2, name=f"y1_{m}")
                nc.scalar.copy(out=y1[:], in_=py1[:, 0:DMID])
                nc.sync.dma_start(out=ag1_in[m * P:(m + 1) * P, :], in_=y1[:])

        nc.gpsimd.collective_compute(
            "AllGather", mybir.AluOpType.bypass, replica_groups=rg,
            ins=[ag1_in.ap().opt()], outs=[ag1_out.ap().opt()],
        )

        if kphases >= 2:
            # ---- phase B: hT = relu(A @ Y1).T, feature-major [256, 1024] ---
            ht0 = sb.tile([P, R], BF16, tag="ht0", bufs=1)
            ht1 = sb.tile([P, R], BF16, tag="ht1", bufs=1)
            with tc.tile_pool(name="pb", bufs=1) as pb:
                for b in range(NBLK):
                    k0 = b * b_ch
                    g1 = pb.tile([P, b_ch, DMID], BF16, tag="g1", bufs=8,
                                 name=f"g1_{b}")
                    nc.gpsimd.dma_gather(
                        out_ap=g1[:],
                        in_ap=ag1_out[:, :],
                        idxs_ap=idx_t[:, k0 * 8:(k0 + b_ch) * 8],
                        num_idxs=b_ch * P,
                        num_idxs_reg=b_ch * P,
                        elem_size=DMID,
                        single_packet=False,
                        queue_num=b % 4,
                    )
                    pha = bank1(f"pha_{b}")
                    phb = bank1(f"phb_{b}")
                    for j in range(b_ch):
                        k = k0 + j
                        nc.tensor.matmul(
                            out=pha[:, 0:P],
                            lhsT=g1[:, j, 0:P],
                            rhs=s_all[:, k * P:(k + 1) * P],
                            start=(j == 0), stop=(j == b_ch - 1),
                        )
                    for j in range(b_ch):
                        k = k0 + j
                        nc.tensor.matmul(
                            out=phb[:, 0:P],
                            lhsT=g1[:, j, P:2 * P],
                            rhs=s_all[:, k * P:(k + 1) * P],
                            start=(j == 0), stop=(j == b_ch - 1),
                        )
                    nc.scalar.activation(
                        out=ht0[:, b * P:(b + 1) * P], in_=pha[:, 0:P],
                        func=mybir.ActivationFunctionType.Relu,
                    )
                    nc.scalar.activation(
                        out=ht1[:, b * P:(b + 1) * P], in_=phb[:, 0:P],
                        func=mybir.ActivationFunctionType.Relu,
                    )
                    # phase C fused: Y2 block -> ag2_in right after relu
                    py2 = bank1(f"py2_{b}")
                    nc.tensor.matmul(
                        out=py2[:, 0:DEMB], lhsT=ht0[:, b * P:(b + 1) * P],
                        rhs=w2a[:], start=True, stop=False,
                    )
                    nc.tensor.matmul(
                        out=py2[:, 0:DEMB], lhsT=ht1[:, b * P:(b + 1) * P],
                        rhs=w2b[:], start=False, stop=True,
                    )
                    y2 = pb.tile([P, Y2W], BF16, tag="y2", bufs=2, name=f"y2_{b}")
                    nc.scalar.copy(out=y2[:, 0:DEMB], in_=py2[:, 0:DEMB])
                    nc.vector.memset(y2[:, DEMB:Y2W], 0)
                    nc.sync.dma_start(out=ag2_in[b * P:(b + 1) * P, :], in_=y2[:])

        if kphases >= 3:
            nc.gpsimd.collective_compute(
                "AllGather", mybir.AluOpType.bypass, replica_groups=rg,
                ins=[ag2_in.ap().opt()], outs=[ag2_out.ap().opt()],
            )

        if kphases >= 4:
            # ---- phase D: embT = (A @ Y2).T; c_loc = [embT; -sqh; -sql] ----
            c_loc = sb.tile([DEMB, R], BF16, tag="cloc", bufs=1)
            lhsT_all = sb.tile([DEMB + 2, R], BF16, tag="lhsT", bufs=1)
            bias_t = sb.tile([P, NBLK], F32, tag="bias", bufs=1)
            with tc.tile_pool(name="pd", bufs=1) as pd:
                sqtmp = pd.tile([DEMB, P], F32, tag="sqtmp", bufs=1)
                negones = pd.tile([DEMB, 1], F32, tag="negones", bufs=1)
                nc.vector.memset(negones[:], -1.0)
                one1 = pd.tile([1, 1], F32, tag="one1", bufs=1)
                nc.vector.memset(one1[:], 1.0)
                sqrow = pd.tile([1, R], F32, tag="sqrow", bufs=1)
                sqh_b = pd.tile([1, R], BF16, tag="sqh_b", bufs=1)
                sql_b = pd.tile([1, R], BF16, tag="sql_b", bufs=1)
                for b in range(NBLK):
                    k0 = b * b_ch
                    g2 = pd.tile([P, b_ch, Y2W], BF16, tag="g2", bufs=8,
                                 name=f"g2_{b}")
                    nc.gpsimd.dma_gather(
                        out_ap=g2[:],
                        in_ap=ag2_out[:, :],
                        idxs_ap=idx_t[:, k0 * 8:(k0 + b_ch) * 8],
                        num_idxs=b_ch * P,
                        num_idxs_reg=b_ch * P,
                        elem_size=Y2W,
                        single_packet=False,
                        queue_num=b % 4,
                    )
                    pe = bank1(f"pe_{b}")
                    for j in range(b_ch):
                        k = k0 + j
                        nc.tensor.matmul(
                            out=pe[0:DEMB, 0:P],
                            lhsT=g2[:, j, 0:DEMB],
                            rhs=s_all[:, k * P:(k + 1) * P],
                            start=(j == 0), stop=(j == b_ch - 1),
                        )
                    bsl = slice(b * P, (b + 1) * P)
                    nc.scalar.copy(out=c_loc[0:DEMB, bsl], in_=pe[0:DEMB, 0:P])
                    # per-block tail, hidden behind the remaining gathers:
                    # lhsT rows, embT^2, -sq, bf16 hi/lo split, bias transpose
                    nc.vector.tensor_scalar_mul(lhsT_all[0:DEMB, bsl],
                                                c_loc[0:DEMB, bsl], 2.0)
                    nc.scalar.square(out=sqtmp[:, 0:P],
                                     in_=c_loc[0:DEMB, bsl])
                    psq = bank1(f"psq_{b}")
                    nc.tensor.matmul(
                        out=psq[0:1, 0:P],
                        lhsT=negones[:],
                        rhs=sqtmp[:, 0:P],
                        start=True, stop=True,
                    )
                    nc.scalar.copy(out=sqrow[0:1, bsl], in_=psq[0:1, 0:P])
                    nc.scalar.copy(out=sqh_b[0:1, bsl], in_=psq[0:1, 0:P])
                    nc.vector.tensor_tensor(
                        out=sql_b[0:1, bsl],
                        in0=psq[0:1, 0:P],
                        in1=sqh_b[0:1, bsl],
                        op=mybir.AluOpType.subtract,
                    )
                    pbt = bank1(f"pbt_{b}")
                    nc.tensor.matmul(
                        out=pbt[:, 0:1], lhsT=sqrow[0:1, bsl],
                        rhs=one1[:], start=True, stop=True,
                    )
                    nc.vector.tensor_copy(bias_t[:, b:b + 1], pbt[:, 0:1])

                nc.sync.dma_start(out=ag3_in[0:DEMB, :], in_=c_loc[:])
                nc.sync.dma_start(out=ag3_in[DEMB:DEMB + 1, :], in_=sqh_b[:])
                nc.sync.dma_start(out=ag3_in[DEMB + 1:DEMB + 2, :], in_=sql_b[:])
                nc.gpsimd.collective_compute(
                    "AllGather", mybir.AluOpType.bypass, replica_groups=rg,
                    ins=[ag3_in.ap().opt()], outs=[ag3_out.ap().opt()],
                )
                nc.vector.memset(lhsT_all[DEMB:DEMB + 2, :], 1.0)

        ps.release()
        if kphases >= 5:
            # ---- phase E: stripes of softmax(-dist) ------------------------
            NQ = 4          # chunks per stripe
            QW = N // NQ    # 2048 cols per chunk (2 PSUM banks x 2 bufs)
            with (
                tc.tile_pool(name="pef", bufs=1) as pef,
                tc.tile_pool(name="pse", bufs=1, space="PSUM") as pse,
            ):
                rhs_full = pef.tile([DEMB + 2, N], BF16, tag="rhs", bufs=1)
                for r in range(NC):
                    nc.sync.dma_start(
                        out=rhs_full[:, r * R:(r + 1) * R],
                        in_=ag3_out[r * (DEMB + 2):(r + 1) * (DEMB + 2), :],
                    )
                for s in range(NBLK):
                    stripe = pef.tile([P, N], F32, tag="stripe", bufs=4,
                                      name=f"stripe_{s}")
                    sums = pef.tile([P, NQ], F32, tag="sums", bufs=2,
                                    name=f"sums_{s}")
                    for q in range(NQ):
                        pz = pse.tile([P, QW], F32, tag="pz", bufs=2,
                                      name=f"pz_{s}_{q}")
                        for jj in range(QW // 512):
                            c0 = q * QW + jj * 512
                            nc.tensor.matmul(
                                out=pz[:, jj * 512:(jj + 1) * 512],
                                lhsT=lhsT_all[:, s * P:(s + 1) * P],
                                rhs=rhs_full[:, c0:c0 + 512],
                                start=True, stop=True,
                            )
                        nc.scalar.activation(
                            out=stripe[:, q * QW:(q + 1) * QW],
                            in_=pz[:],
                            func=mybir.ActivationFunctionType.Exp,
                            bias=bias_t[:, s:s + 1],
                            scale=1.0,
                            accum_out=sums[:, q:q + 1],
                        )
                    stot = pef.tile([P, 1], F32, tag="stot", bufs=2,
                                    name=f"stot_{s}")
                    nc.vector.tensor_reduce(
                        out=stot[:], in_=sums[:], axis=mybir.AxisListType.X,
                        op=mybir.AluOpType.add,
                    )
                    rec = pef.tile([P, 1], F32, tag="rec", bufs=2,
                                   name=f"rec_{s}")
                    nc.vector.reciprocal(rec[:], stot[:])
                    # normalize + store per quarter so the output DMA starts
                    # as soon as the first quarter is scaled
                    for q in range(NQ):
                        qs = slice(q * QW, (q + 1) * QW)
                        nc.vector.tensor_scalar(
                            out=stripe[:, qs],
                            in0=stripe[:, qs],
                            scalar1=rec[:, 0:1],
                            scalar2=1e-10,
                            op0=mybir.AluOpType.mult,
                            op1=mybir.AluOpType.add,
                        )
                        nc.sync.dma_start(out=out[s * P:(s + 1) * P, qs],
                                          in_=stripe[:, qs])

    nc.finalize()
    return nc


def _block_unique(c, edge_src, edge_dst, edge_val):
    """Per 128-row block: unique src rows + (slot, dstrow, val) triples."""
    out = []
    sel = (edge_dst >= c * R) & (edge_dst < (c + 1) * R)
    src = edge_src[sel]
    val = edge_val[sel]
    loc = edge_dst[sel] - c * R
    blk = loc // P
    for b in range(NBLK):
        m = blk == b
        sb_, vb, rb = src[m], val[m], loc[m] % P
        uniq, slot = np.unique(sb_, return_inverse=True)
        out.append((uniq, slot, rb, vb))
    return out


def _prep_core(blocks, b_ch):
    """Pack per-block unique srcs into b_ch chunks; build multi-hot S."""
    c1 = NBLK * b_ch
    src_pad = np.zeros(c1 * P, np.int16)
    smat = np.zeros((c1 * P, P), np.float32)  # [slot, dstrow]
    for b, (uniq, slot, rb, vb) in enumerate(blocks):
        assert len(uniq) <= b_ch * P
        lo = b * b_ch * P
        src_pad[lo:lo + len(uniq)] = uniq.astype(np.int16)
        np.add.at(smat, (lo + slot, rb), vb)

    idx = np.tile(np.ascontiguousarray(src_pad.reshape(-1, 16).T), (8, 1))
    # device tile layout: [slot%128 partition, chunk*128 + dstrow free]
    smat_d = np.ascontiguousarray(
        smat.reshape(c1, P, P).transpose(1, 0, 2).reshape(P, c1 * P)
    ).astype(BFNP)
    return idx, smat_d


def kernel(X, W1, W2, edge_val, edge_src, edge_dst):
    global LAST_RESULTS
    X = np.asarray(X, np.float32)
    W1 = np.asarray(W1, np.float32)
    W2 = np.asarray(W2, np.float32)
    edge_val = np.asarray(edge_val, np.float32)
    edge_src = np.asarray(edge_src, np.int32)
    edge_dst = np.asarray(edge_dst, np.int32)

    # chunks per (core, block) from deduped src counts, uniform across cores
    per_core_blocks = [_block_unique(c, edge_src, edge_dst, edge_val)
                       for c in range(NC)]
    b_ch = max(int(np.ceil(len(u) / P)) for blocks in per_core_blocks
               for (u, _, _, _) in blocks)
    b_ch = max(b_ch, 1)

    if b_ch not in _GRAPH_CACHE:
        _GRAPH_CACHE[b_ch] = _build(b_ch)
    nc = _GRAPH_CACHE[b_ch]

    w1b = W1.astype(BFNP)
    w2b = W2.astype(BFNP)
    in_maps = []
    for c in range(NC):
        idx, smat_d = _prep_core(per_core_blocks[c], b_ch)
        in_maps.append({
            "xT": np.ascontiguousarray(X[c * R:(c + 1) * R].T).astype(BFNP),
            "w1": w1b,
            "w2": w2b,
            "idx": idx,
            "smat": smat_d,
        })

    trace = os.environ.get("KERNEL_TRACE", "0") == "1"
    res = run_bass_kernel_spmd(nc, in_maps, core_ids=list(range(NC)), trace=trace)
    LAST_RESULTS = res
    return np.concatenate([res.results[c]["out"] for c in range(NC)], axis=0)



# revision 4
# speedup vs baseline: 1.0351x; 1.0351x over previous
"""AdaGAE forward on 8 Trainium2 NeuronCores.

reference:
    h   = relu(spmm(X @ W1))        # spmm = COO Laplacian x dense
    emb = spmm(h @ W2)
    out = softmax(-pairwise_sqdist(emb), axis=1) + 1e-10   # [8192, 8192]

Sharding: nodes row-sharded, 1024 rows/core. Each spmm gathers rows of the
all-gathered table with dma_gather and reduces per-128-edge chunks with a
one-hot matmul G.T @ S that also transposes (h and emb are kept feature-major).
All matmuls run in bf16 (f32 PSUM accumulate); sq is computed from the SAME
quantized embedding so the softmax stays consistent. The NxN phase computes
z = 2*emb_i.emb_j - sq_j via one K=66 bf16 matmul per [128,512] block
(-sq_j folded in as two extra contraction rows, bf16 hi+lo), then
exp(z - sq_i) on ScalarE (row max of -dist is exactly 0, bias=-sq_i in f32)
writing a bf16 stripe, row sums on VectorE, one VectorE normalize
(*1/s + 1e-10) pass, DMA out in bf16 (host converts to f32).

Perf notes vs v0 (384us):
 - output stored bf16 (halves the 33.5MB/core output DMA)
 - tiny warmup AllGather at t=0 absorbs ncfw ring-setup (~25us) off AG1
 - filler matmuls across the 3 collective windows keep the PE HAM clock
   at 2.4GHz (otherwise nearly the whole kernel ran at K=4 = 1.2GHz)
 - phase A runs k-outer so matmuls start as soon as the first k-chunk
   DMA lands; s_all (2MB) is loaded after the X chunks
 - phases B/D prefetch all 8 dma_gathers up front and run the one-hot
   matmuls back-to-back; dependent ScalarE/TensorE tails are deferred
 - phase E row sums moved off ScalarE (accum_out) onto VectorE reduces
"""

import os
import sys

if "/opt/trn_rl_repo" not in sys.path:
    sys.path.insert(0, "/opt/trn_rl_repo")

import ml_dtypes
import numpy as np

import concourse.bacc as bacc
import concourse.bass as bass
import concourse.mybir as mybir
from concourse.tile import TileContext
from concourse.bass_utils import run_bass_kernel_spmd

NC = 8          # cores
N = 8192        # nodes
R = N // NC     # rows per core
P = 128
DIN = 1024
DMID = 256
DEMB = 64
KC = DIN // P   # k-chunks for X @ W1
NBLK = R // P   # 128-row blocks per core
Y2W = 128       # Y2 table row padded to 128 bf16 cols (gather needs 256B rows)

F32 = mybir.dt.float32
BF16 = mybir.dt.bfloat16
I16 = mybir.dt.int16
BFNP = ml_dtypes.bfloat16

LAST_RESULTS = None  # BassKernelResults of the most recent run (for test.py)

_GRAPH_CACHE = {}

# filler matmuls (512 cols each, ~210-430ns) to span each collective window
W_AG1 = int(os.environ.get("W_AG1", "110"))
W_AG2 = int(os.environ.get("W_AG2", "70"))
W_AG3 = int(os.environ.get("W_AG3", "56"))


def _build(b_ch: int):
    """Build the per-core Bass graph. b_ch = gather chunks per 128-row block."""
    kphases = int(os.environ.get("KPHASES", "9"))
    c1 = NBLK * b_ch          # total chunks per core
    ne = c1 * P               # padded edges per core
    nc = bacc.Bacc(None, target_bir_lowering=False, debug=False, num_devices=NC,
                   num_swdge_queues=4)

    xT = nc.declare_dram_parameter("xT", [DIN, R], BF16, isOutput=False)
    w1 = nc.declare_dram_parameter("w1", [DIN, DMID], BF16, isOutput=False)
    w2 = nc.declare_dram_parameter("w2", [DMID, DEMB], BF16, isOutput=False)
    idx = nc.declare_dram_parameter("idx", [128, ne // 16], I16, isOutput=False)
    smat = nc.declare_dram_parameter("smat", [128, c1 * P], BF16, isOutput=False)
    out = nc.declare_dram_parameter("out", [R, N], BF16, isOutput=True)

    ag0_in = nc.dram_tensor("ag0_in", [1, 512], BF16)
    ag0_out = nc.dram_tensor("ag0_out", [NC, 512], BF16, addr_space="Shared")
    ag1_in = nc.dram_tensor("ag1_in", [R, DMID], BF16)
    ag1_out = nc.dram_tensor("ag1_out", [N, DMID], BF16, addr_space="Shared")
    ag2_in = nc.dram_tensor("ag2_in", [R, Y2W], BF16)
    ag2_out = nc.dram_tensor("ag2_out", [N, Y2W], BF16, addr_space="Shared")
    ag3_in = nc.dram_tensor("ag3_in", [DEMB + 2, R], BF16)
    ag3_out = nc.dram_tensor("ag3_out", [NC * (DEMB + 2), R], BF16,
                             addr_space="Shared")

    rg = [list(range(NC))]

    with (
        TileContext(nc) as tc,
        tc.tile_pool(name="sb", bufs=1) as sb,
    ):
        # warmup collective: absorbs ncfw ring setup during the input DMAs
        nc.gpsimd.collective_compute(
            "AllGather", mybir.AluOpType.bypass, replica_groups=rg,
            ins=[ag0_in.ap().opt()], outs=[ag0_out.ap().opt()],
        )

        # ---- phase A: Y1 = X @ W1 (row-sharded) -> ag1_in ------------------
        # k-outer over 2 halves of 4 blocks so matmuls start on chunk 0
        with (
            tc.tile_pool(name="pa", bufs=1) as pa,
            tc.tile_pool(name="pap", bufs=1, space="PSUM") as pap,
        ):
            xts, w1s = [], []
            for k in range(KC):
                xt_t = pa.tile([P, R], BF16, tag="xt", bufs=KC, name=f"xt_{k}")
                nc.sync.dma_start(out=xt_t[:], in_=xT[k * P:(k + 1) * P, :])
                xts.append(xt_t)
                w1_t = pa.tile([P, DMID], BF16, tag="w1", bufs=KC, name=f"w1_{k}")
                nc.sync.dma_start(out=w1_t[:], in_=w1[k * P:(k + 1) * P, :])
                w1s.append(w1_t)

            # persistent small loads next, the 2MB s_all last (only phase B
            # needs it; keeps it off phase A's critical DMA path)
            idx_t = sb.tile([128, ne // 16], I16, tag="idx", bufs=1)
            nc.sync.dma_start(out=idx_t[:], in_=idx[:, :])
            w2a = sb.tile([P, DEMB], BF16, tag="w2a", bufs=1)
            nc.sync.dma_start(out=w2a[:], in_=w2[0:P, :])
            w2b = sb.tile([P, DEMB], BF16, tag="w2b", bufs=1)
            nc.sync.dma_start(out=w2b[:], in_=w2[P:2 * P, :])
            s_all = sb.tile([P, c1 * P], BF16, tag="sall", bufs=1)
            nc.sync.dma_start(out=s_all[:], in_=smat[:, :])

            for half in range(2):
                ms = range(half * 4, half * 4 + 4)
                py1s = {m: pap.tile([P, DMID], F32, tag="py1", bufs=8,
                                    name=f"py1_{m}") for m in ms}
                for k in range(KC):
                    for m in ms:
                        nc.tensor.matmul(
                            out=py1s[m][:],
                            lhsT=xts[k][:, m * P:(m + 1) * P],
                            rhs=w1s[k][:],
                            start=(k == 0),
                            stop=(k == KC - 1),
                        )
                for m in ms:
                    y1 = pa.tile([P, DMID], BF16, tag="y1", bufs=2,
                                 name=f"y1_{m}")
                    nc.scalar.copy(out=y1[:], in_=py1s[m][:])
                    nc.sync.dma_start(out=ag1_in[m * P:(m + 1) * P, :],
                                      in_=y1[:])

        nc.gpsimd.collective_compute(
            "AllGather", mybir.AluOpType.bypass, replica_groups=rg,
            ins=[ag1_in.ap().opt()], outs=[ag1_out.ap().opt()],
        )

        ps = tc.alloc_tile_pool(name="ps", bufs=1, space="PSUM")

        def bank1(name):
            return ps.tile([P, 512], F32, tag="bank1", bufs=6, name=name)

        def warm(n, tag):
            # filler matmuls: keep the PE HAM clock released (K=8) across a
            # collective window. Reads are of already-resident tiles; the
            # output bank cycles through the bank1 ring like any other tile.
            for i in range(n):
                dps = bank1(f"warm_{tag}_{i}")
                nc.tensor.matmul(
                    out=dps[0:DEMB, 0:512],
                    lhsT=w2a[:],
                    rhs=s_all[:, 0:512],
                    start=True, stop=True,
                )

        warm(W_AG1, "ag1")

        if kphases >= 2:
            # ---- phase B: hT = relu(A @ Y1).T, feature-major [256, 1024] ---
            ht0 = sb.tile([P, R], BF16, tag="ht0", bufs=1)
            ht1 = sb.tile([P, R], BF16, tag="ht1", bufs=1)
            with tc.tile_pool(name="pb", bufs=1) as pb:
                g1s = []
                for b in range(NBLK):
                    k0 = b * b_ch
                    g1 = pb.tile([P, b_ch, DMID], BF16, tag="g1", bufs=8,
                                 name=f"g1_{b}")
                    nc.gpsimd.dma_gather(
                        out_ap=g1[:],
                        in_ap=ag1_out[:, :],
                        idxs_ap=idx_t[:, k0 * 8:(k0 + b_ch) * 8],
                        num_idxs=b_ch * P,
                        num_idxs_reg=b_ch * P,
                        elem_size=DMID,
                        single_packet=False,
                        queue_num=b % 4,
                    )
                    g1s.append(g1)
                for b in range(NBLK):
                    k0 = b * b_ch
                    g1 = g1s[b]
                    pha = bank1(f"pha_{b}")
                    phb = bank1(f"phb_{b}")
                    for j in range(b_ch):
                        k = k0 + j
                        nc.tensor.matmul(
                            out=pha[:, 0:P],
                            lhsT=g1[:, j, 0:P],
                            rhs=s_all[:, k * P:(k + 1) * P],
                            start=(j == 0), stop=(j == b_ch - 1),
                        )
                    for j in range(b_ch):
                        k = k0 + j
                        nc.tensor.matmul(
                            out=phb[:, 0:P],
                            lhsT=g1[:, j, P:2 * P],
                            rhs=s_all[:, k * P:(k + 1) * P],
                            start=(j == 0), stop=(j == b_ch - 1),
                        )
                    nc.scalar.activation(
                        out=ht0[:, b * P:(b + 1) * P], in_=pha[:, 0:P],
                        func=mybir.ActivationFunctionType.Relu,
                    )
                    nc.scalar.activation(
                        out=ht1[:, b * P:(b + 1) * P], in_=phb[:, 0:P],
                        func=mybir.ActivationFunctionType.Relu,
                    )
                # phase C: Y2 blocks -> ag2_in (deferred so the spmm matmul
                # stream above runs back-to-back on TensorE)
                for b in range(NBLK):
                    py2 = bank1(f"py2_{b}")
                    nc.tensor.matmul(
                        out=py2[:, 0:DEMB], lhsT=ht0[:, b * P:(b + 1) * P],
                        rhs=w2a[:], start=True, stop=False,
                    )
                    nc.tensor.matmul(
                        out=py2[:, 0:DEMB], lhsT=ht1[:, b * P:(b + 1) * P],
                        rhs=w2b[:], start=False, stop=True,
                    )
                    y2 = pb.tile([P, Y2W], BF16, tag="y2", bufs=2, name=f"y2_{b}")
                    nc.scalar.copy(out=y2[:, 0:DEMB], in_=py2[:, 0:DEMB])
                    nc.vector.memset(y2[:, DEMB:Y2W], 0)
                    nc.sync.dma_start(out=ag2_in[b * P:(b + 1) * P, :], in_=y2[:])

        if kphases >= 3:
            nc.gpsimd.collective_compute(
                "AllGather", mybir.AluOpType.bypass, replica_groups=rg,
                ins=[ag2_in.ap().opt()], outs=[ag2_out.ap().opt()],
            )

        warm(W_AG2, "ag2")

        if kphases >= 4:
            # ---- phase D: embT = (A @ Y2).T; c_loc = [embT; -sqh; -sql] ----
            c_loc = sb.tile([DEMB, R], BF16, tag="cloc", bufs=1)
            lhsT_all = sb.tile([DEMB + 2, R], BF16, tag="lhsT", bufs=1)
            bias_t = sb.tile([P, NBLK], F32, tag="bias", bufs=1)
            with tc.tile_pool(name="pd", bufs=1) as pd:
                negones = pd.tile([DEMB, 1], F32, tag="negones", bufs=1)
                nc.vector.memset(negones[:], -1.0)
                one1 = pd.tile([1, 1], F32, tag="one1", bufs=1)
                nc.vector.memset(one1[:], 1.0)
                sqrow = pd.tile([1, R], F32, tag="sqrow", bufs=1)
                sqh_b = pd.tile([1, R], BF16, tag="sqh_b", bufs=1)
                sql_b = pd.tile([1, R], BF16, tag="sql_b", bufs=1)
                g2s = []
                for b in range(NBLK):
                    k0 = b * b_ch
                    g2 = pd.tile([P, b_ch, Y2W], BF16, tag="g2", bufs=8,
                                 name=f"g2_{b}")
                    nc.gpsimd.dma_gather(
                        out_ap=g2[:],
                        in_ap=ag2_out[:, :],
                        idxs_ap=idx_t[:, k0 * 8:(k0 + b_ch) * 8],
                        num_idxs=b_ch * P,
                        num_idxs_reg=b_ch * P,
                        elem_size=Y2W,
                        single_packet=False,
                        queue_num=b % 4,
                    )
                    g2s.append(g2)
                for b in range(NBLK):
                    k0 = b * b_ch
                    g2 = g2s[b]
                    pe = bank1(f"pe_{b}")
                    for j in range(b_ch):
                        k = k0 + j
                        nc.tensor.matmul(
                            out=pe[0:DEMB, 0:P],
                            lhsT=g2[:, j, 0:DEMB],
                            rhs=s_all[:, k * P:(k + 1) * P],
                            start=(j == 0), stop=(j == b_ch - 1),
                        )
                    bsl = slice(b * P, (b + 1) * P)
                    nc.scalar.copy(out=c_loc[0:DEMB, bsl], in_=pe[0:DEMB, 0:P])
                # deferred per-block tails: lhsT rows, embT^2, -sq, bf16
                # hi/lo split, bias transpose
                for b in range(NBLK):
                    bsl = slice(b * P, (b + 1) * P)
                    sqt = pd.tile([DEMB, P], F32, tag="sqtmp", bufs=2,
                                  name=f"sqt_{b}")
                    nc.vector.tensor_scalar_mul(lhsT_all[0:DEMB, bsl],
                                                c_loc[0:DEMB, bsl], 2.0)
                    nc.scalar.square(out=sqt[:, 0:P],
                                     in_=c_loc[0:DEMB, bsl])
                    psq = bank1(f"psq_{b}")
                    nc.tensor.matmul(
                        out=psq[0:1, 0:P],
                        lhsT=negones[:],
                        rhs=sqt[:, 0:P],
                        start=True, stop=True,
                    )
                    nc.scalar.copy(out=sqrow[0:1, bsl], in_=psq[0:1, 0:P])
                    nc.scalar.copy(out=sqh_b[0:1, bsl], in_=psq[0:1, 0:P])
                    nc.vector.tensor_tensor(
                        out=sql_b[0:1, bsl],
                        in0=psq[0:1, 0:P],
                        in1=sqh_b[0:1, bsl],
                        op=mybir.AluOpType.subtract,
                    )
                    pbt = bank1(f"pbt_{b}")
                    nc.tensor.matmul(
                        out=pbt[:, 0:1], lhsT=sqrow[0:1, bsl],
                        rhs=one1[:], start=True, stop=True,
                    )
                    nc.vector.tensor_copy(bias_t[:, b:b + 1], pbt[:, 0:1])

                nc.sync.dma_start(out=ag3_in[0:DEMB, :], in_=c_loc[:])
                nc.sync.dma_start(out=ag3_in[DEMB:DEMB + 1, :], in_=sqh_b[:])
                nc.sync.dma_start(out=ag3_in[DEMB + 1:DEMB + 2, :], in_=sql_b[:])
                nc.gpsimd.collective_compute(
                    "AllGather", mybir.AluOpType.bypass, replica_groups=rg,
                    ins=[ag3_in.ap().opt()], outs=[ag3_out.ap().opt()],
                )
                nc.vector.memset(lhsT_all[DEMB:DEMB + 2, :], 1.0)
                warm(W_AG3, "ag3")

        ps.release()
        if kphases >= 5:
            # ---- phase E: stripes of softmax(-dist) ------------------------
            NQ = 4          # chunks per stripe
            QW = N // NQ    # 2048 cols per chunk (2 PSUM banks x 2 bufs)
            with (
                tc.tile_pool(name="pef", bufs=1) as pef,
                tc.tile_pool(name="pse", bufs=1, space="PSUM") as pse,
            ):
                rhs_full = pef.tile([DEMB + 2, N], BF16, tag="rhs", bufs=1)
                for r in range(NC):
                    nc.sync.dma_start(
                        out=rhs_full[:, r * R:(r + 1) * R],
                        in_=ag3_out[r * (DEMB + 2):(r + 1) * (DEMB + 2), :],
                    )
                for s in range(NBLK):
                    stripe = pef.tile([P, N], BF16, tag="stripe", bufs=4,
                                      name=f"stripe_{s}")
                    sums = pef.tile([P, NQ], F32, tag="sums", bufs=2,
                                    name=f"sums_{s}")
                    for q in range(NQ):
                        pz = pse.tile([P, QW], F32, tag="pz", bufs=2,
                                      name=f"pz_{s}_{q}")
                        for jj in range(QW // 512):
                            c0 = q * QW + jj * 512
                            nc.tensor.matmul(
                                out=pz[:, jj * 512:(jj + 1) * 512],
                                lhsT=lhsT_all[:, s * P:(s + 1) * P],
                                rhs=rhs_full[:, c0:c0 + 512],
                                start=True, stop=True,
                            )
                        nc.scalar.activation(
                            out=stripe[:, q * QW:(q + 1) * QW],
                            in_=pz[:],
                            func=mybir.ActivationFunctionType.Exp,
                            bias=bias_t[:, s:s + 1],
                            scale=1.0,
                        )
                        nc.vector.tensor_reduce(
                            out=sums[:, q:q + 1],
                            in_=stripe[:, q * QW:(q + 1) * QW],
                            axis=mybir.AxisListType.X,
                            op=mybir.AluOpType.add,
                        )
                    stot = pef.tile([P, 1], F32, tag="stot", bufs=2,
                                    name=f"stot_{s}")
                    nc.vector.tensor_reduce(
                        out=stot[:], in_=sums[:], axis=mybir.AxisListType.X,
                        op=mybir.AluOpType.add,
                    )
                    rec = pef.tile([P, 1], F32, tag="rec", bufs=2,
                                   name=f"rec_{s}")
                    nc.vector.reciprocal(rec[:], stot[:])
                    # normalize + store per quarter so the output DMA starts
                    # as soon as the first quarter is scaled
                    for q in range(NQ):
                        qs = slice(q * QW, (q + 1) * QW)
                        nc.vector.tensor_scalar(
                            out=stripe[:, qs],
                            in0=stripe[:, qs],
                            scalar1=rec[:, 0:1],
                            scalar2=1e-10,
                            op0=mybir.AluOpType.mult,
                            op1=mybir.AluOpType.add,
                        )
                        nc.sync.dma_start(out=out[s * P:(s + 1) * P, qs],
                                          in_=stripe[:, qs])

    nc.finalize()
    return nc


def _block_unique(c, edge_src, edge_dst, edge_val):
    """Per 128-row block: unique src rows + (slot, dstrow, val) triples."""
    out = []
    sel = (edge_dst >= c * R) & (edge_dst < (c + 1) * R)
    src = edge_src[sel]
    val = edge_val[sel]
    loc = edge_dst[sel] - c * R
    blk = loc // P
    for b in range(NBLK):
        m = blk == b
        sb_, vb, rb = src[m], val[m], loc[m] % P
        uniq, slot = np.unique(sb_, return_inverse=True)
        out.append((uniq, slot, rb, vb))
    return out


def _prep_core(blocks, b_ch):
    """Pack per-block unique srcs into b_ch chunks; build multi-hot S."""
    c1 = NBLK * b_ch
    src_pad = np.zeros(c1 * P, np.int16)
    smat = np.zeros((c1 * P, P), np.float32)  # [slot, dstrow]
    for b, (uniq, slot, rb, vb) in enumerate(blocks):
        assert len(uniq) <= b_ch * P
        lo = b * b_ch * P
        src_pad[lo:lo + len(uniq)] = uniq.astype(np.int16)
        np.add.at(smat, (lo + slot, rb), vb)

    idx = np.tile(np.ascontiguousarray(src_pad.reshape(-1, 16).T), (8, 1))
    # device tile layout: [slot%128 partition, chunk*128 + dstrow free]
    smat_d = np.ascontiguousarray(
        smat.reshape(c1, P, P).transpose(1, 0, 2).reshape(P, c1 * P)
    ).astype(BFNP)
    return idx, smat_d


def kernel(X, W1, W2, edge_val, edge_src, edge_dst):
    global LAST_RESULTS
    X = np.asarray(X, np.float32)
    W1 = np.asarray(W1, np.float32)
    W2 = np.asarray(W2, np.float32)
    edge_val = np.asarray(edge_val, np.float32)
    edge_src = np.asarray(edge_src, np.int32)
    edge_dst = np.asarray(edge_dst, np.int32)

    # chunks per (core, block) from deduped src counts, uniform across cores
    per_core_blocks = [_block_unique(c, edge_src, edge_dst, edge_val)
                       for c in range(NC)]
    b_ch = max(int(np.ceil(len(u) / P)) for blocks in per_core_blocks
               for (u, _, _, _) in blocks)
    b_ch = max(b_ch, 1)

    if b_ch not in _GRAPH_CACHE:
        _GRAPH_CACHE[b_ch] = _build(b_ch)
    nc = _GRAPH_CACHE[b_ch]

    w1b = W1.astype(BFNP)
    w2b = W2.astype(BFNP)
    in_maps = []
    for c in range(NC):
        idx, smat_d = _prep_core(per_core_blocks[c], b_ch)
        in_maps.append({
            "xT": np.ascontiguousarray(X[c * R:(c + 1) * R].T).astype(BFNP),
            "w1": w1b,
            "w2": w2b,
            "idx": idx,
            "smat": smat_d,
        })

    trace = os.environ.get("KERNEL_TRACE", "0") == "1"
    res = run_bass_kernel_spmd(nc, in_maps, core_ids=list(range(NC)), trace=trace)
    LAST_RESULTS = res
    return np.concatenate(
        [res.results[c]["out"] for c in range(NC)], axis=0
    ).astype(np.float32)


# revision 11
# speedup vs baseline: 1.0778x; 1.0413x over previous
"""AdaGAE forward on 8 Trainium2 NeuronCores.

reference:
    h   = relu(spmm(X @ W1))        # spmm = COO Laplacian x dense
    emb = spmm(h @ W2)
    out = softmax(-pairwise_sqdist(emb), axis=1) + 1e-10   # [8192, 8192]

Sharding: nodes row-sharded, 1024 rows/core. Each spmm gathers rows of the
all-gathered table with dma_gather and reduces per-128-edge chunks with a
one-hot matmul G.T @ S that also transposes (h and emb are kept feature-major).
All matmuls run in bf16 (f32 PSUM accumulate); sq is computed from the SAME
quantized embedding so the softmax stays consistent. The NxN phase computes
z = 2*emb_i.emb_j - sq_j via one K=66 bf16 matmul per [128,512] block
(-sq_j folded in as two extra contraction rows, bf16 hi+lo), then
exp(z - sq_i) on ScalarE (row max of -dist is exactly 0, bias=-sq_i in f32)
writing a bf16 stripe, row sums on VectorE, one VectorE normalize
(*1/s + 1e-10) pass, DMA out in bf16 (host converts to f32).

Perf notes vs v0 (384us):
 - output stored bf16 (halves the 33.5MB/core output DMA)
 - tiny warmup AllGather at t=0 absorbs ncfw ring-setup (~25us) off AG1
 - filler matmuls across the 3 collective windows keep the PE HAM clock
   at 2.4GHz (otherwise nearly the whole kernel ran at K=4 = 1.2GHz)
 - phase A runs k-outer so matmuls start as soon as the first k-chunk
   DMA lands; s_all (2MB) is loaded after the X chunks
 - phases B/D prefetch all 8 dma_gathers up front and run the one-hot
   matmuls back-to-back; dependent ScalarE/TensorE tails are deferred
 - phase E row sums moved off ScalarE (accum_out) onto VectorE reduces
"""

import os
import sys

if "/opt/trn_rl_repo" not in sys.path:
    sys.path.insert(0, "/opt/trn_rl_repo")

import ml_dtypes
import numpy as np

import concourse.bacc as bacc
import concourse.bass as bass
import concourse.mybir as mybir
from concourse.tile import TileContext
from concourse.bass_utils import run_bass_kernel_spmd

NC = 8          # cores
N = 8192        # nodes
R = N // NC     # rows per core
P = 128
DIN = 1024
DMID = 256
DEMB = 64
KC = DIN // P   # k-chunks for X @ W1
NBLK = R // P   # 128-row blocks per core
Y2W = 128       # Y2 table row padded to 128 bf16 cols (gather needs 256B rows)

F32 = mybir.dt.float32
BF16 = mybir.dt.bfloat16
I16 = mybir.dt.int16
BFNP = ml_dtypes.bfloat16

LAST_RESULTS = None  # BassKernelResults of the most recent run (for test.py)

_GRAPH_CACHE = {}

# filler matmuls after each collective to re-warm the PE clock before the
# next phase's matmul stream (sized to the gather lead-in window)
W_AG1 = int(os.environ.get("W_AG1", "90"))
W_AG2 = int(os.environ.get("W_AG2", "60"))
W_AG3 = int(os.environ.get("W_AG3", "40"))


def _build(b_ch: int):
    """Build the per-core Bass graph. b_ch = gather chunks per 128-row block."""
    kphases = int(os.environ.get("KPHASES", "9"))
    c1 = NBLK * b_ch          # total chunks per core
    ne = c1 * P               # padded edges per core
    nc = bacc.Bacc(None, target_bir_lowering=False, debug=False, num_devices=NC,
                   num_swdge_queues=4)

    xT = nc.declare_dram_parameter("xT", [DIN, R], BF16, isOutput=False)
    w1 = nc.declare_dram_parameter("w1", [DIN, DMID], BF16, isOutput=False)
    w2 = nc.declare_dram_parameter("w2", [DMID, DEMB], BF16, isOutput=False)
    idx = nc.declare_dram_parameter("idx", [128, ne // 16], I16, isOutput=False)
    smat = nc.declare_dram_parameter("smat", [128, c1 * P], BF16, isOutput=False)
    out = nc.declare_dram_parameter("out", [R, N], BF16, isOutput=True)

    ag1_in = nc.dram_tensor("ag1_in", [R, DMID], BF16)
    ag1_out = nc.dram_tensor("ag1_out", [N, DMID], BF16, addr_space="Shared")
    ag2_in = nc.dram_tensor("ag2_in", [R, Y2W], BF16)
    ag2_out = nc.dram_tensor("ag2_out", [N, Y2W], BF16, addr_space="Shared")
    ag3_in = nc.dram_tensor("ag3_in", [DEMB + 2, R], BF16)
    ag3_out = nc.dram_tensor("ag3_out", [NC * (DEMB + 2), R], BF16,
                             addr_space="Shared")

    rg = [list(range(NC))]

    with (
        TileContext(nc) as tc,
        tc.tile_pool(name="sb", bufs=1) as sb,
    ):
        # ---- phase A: Y1 = X @ W1 (row-sharded) -> ag1_in ------------------
        # k-outer over 2 halves of 4 blocks so matmuls start on chunk 0
        with (
            tc.tile_pool(name="pa", bufs=1) as pa,
            tc.tile_pool(name="pap", bufs=1, space="PSUM") as pap,
        ):
            xts, w1s = [], []
            for k in range(KC):
                xt_t = pa.tile([P, R], BF16, tag="xt", bufs=KC, name=f"xt_{k}")
                nc.sync.dma_start(out=xt_t[:], in_=xT[k * P:(k + 1) * P, :])
                xts.append(xt_t)
                w1_t = pa.tile([P, DMID], BF16, tag="w1", bufs=KC, name=f"w1_{k}")
                nc.sync.dma_start(out=w1_t[:], in_=w1[k * P:(k + 1) * P, :])
                w1s.append(w1_t)

            # persistent small loads next, the 2MB s_all last (only phase B
            # needs it; keeps it off phase A's critical DMA path)
            idx_t = sb.tile([128, ne // 16], I16, tag="idx", bufs=1)
            nc.sync.dma_start(out=idx_t[:], in_=idx[:, :])
            w2a = sb.tile([P, DEMB], BF16, tag="w2a", bufs=1)
            nc.sync.dma_start(out=w2a[:], in_=w2[0:P, :])
            w2b = sb.tile([P, DEMB], BF16, tag="w2b", bufs=1)
            nc.sync.dma_start(out=w2b[:], in_=w2[P:2 * P, :])
            s_all = sb.tile([P, c1 * P], BF16, tag="sall", bufs=1)
            nc.sync.dma_start(out=s_all[:], in_=smat[:, :])

            for half in range(2):
                ms = range(half * 4, half * 4 + 4)
                py1s = {m: pap.tile([P, DMID], F32, tag="py1", bufs=8,
                                    name=f"py1_{m}") for m in ms}
                for k in range(KC):
                    for m in ms:
                        nc.tensor.matmul(
                            out=py1s[m][:],
                            lhsT=xts[k][:, m * P:(m + 1) * P],
                            rhs=w1s[k][:],
                            start=(k == 0),
                            stop=(k == KC - 1),
                        )
                for m in ms:
                    y1 = pa.tile([P, DMID], BF16, tag="y1", bufs=2,
                                 name=f"y1_{m}")
                    nc.scalar.copy(out=y1[:], in_=py1s[m][:])
                    nc.sync.dma_start(out=ag1_in[m * P:(m + 1) * P, :],
                                      in_=y1[:])

        nc.gpsimd.collective_compute(
            "AllGather", mybir.AluOpType.bypass, replica_groups=rg,
            ins=[ag1_in.ap().opt()], outs=[ag1_out.ap().opt()],
        )

        ps = tc.alloc_tile_pool(name="ps", bufs=1, space="PSUM")

        def bank1(name):
            return ps.tile([P, 512], F32, tag="bank1", bufs=6, name=name)

        def warm(n, tag, rhs_tile, kdim, cols):
            # filler matmuls: re-warm the PE HAM clock (K=8) right after a
            # collective completes, bridging the gather lead-in before the
            # next phase's real matmuls. rhs_tile is a small SBUF tile DMA'd
            # from the collective's output, so these cannot start earlier.
            for i in range(n):
                dps = bank1(f"warm_{tag}_{i}")
                nc.tensor.matmul(
                    out=dps[0:DEMB, 0:cols],
                    lhsT=w2a[0:kdim, :],
                    rhs=rhs_tile[0:kdim, 0:cols],
                    start=True, stop=True,
                )

        wt1 = sb.tile([P, DMID], BF16, tag="wt1", bufs=1)
        nc.sync.dma_start(out=wt1[:], in_=ag1_out[0:P, :])
        warm(W_AG1, "ag1", wt1, P, DMID)

        if kphases >= 2:
            # ---- phase B: hT = relu(A @ Y1).T, feature-major [256, 1024] ---
            ht0 = sb.tile([P, R], BF16, tag="ht0", bufs=1)
            ht1 = sb.tile([P, R], BF16, tag="ht1", bufs=1)
            with tc.tile_pool(name="pb", bufs=1) as pb:
                g1s = []
                for b in range(NBLK):
                    k0 = b * b_ch
                    g1 = pb.tile([P, b_ch, DMID], BF16, tag="g1", bufs=8,
                                 name=f"g1_{b}")
                    nc.gpsimd.dma_gather(
                        out_ap=g1[:],
                        in_ap=ag1_out[:, :],
                        idxs_ap=idx_t[:, k0 * 8:(k0 + b_ch) * 8],
                        num_idxs=b_ch * P,
                        num_idxs_reg=b_ch * P,
                        elem_size=DMID,
                        single_packet=False,
                        queue_num=b % 4,
                    )
                    g1s.append(g1)
                for b in range(NBLK):
                    k0 = b * b_ch
                    g1 = g1s[b]
                    pha = bank1(f"pha_{b}")
                    phb = bank1(f"phb_{b}")
                    for j in range(b_ch):
                        k = k0 + j
                        nc.tensor.matmul(
                            out=pha[:, 0:P],
                            lhsT=g1[:, j, 0:P],
                            rhs=s_all[:, k * P:(k + 1) * P],
                            start=(j == 0), stop=(j == b_ch - 1),
                        )
                    for j in range(b_ch):
                        k = k0 + j
                        nc.tensor.matmul(
                            out=phb[:, 0:P],
                            lhsT=g1[:, j, P:2 * P],
                            rhs=s_all[:, k * P:(k + 1) * P],
                            start=(j == 0), stop=(j == b_ch - 1),
                        )
                    nc.scalar.activation(
                        out=ht0[:, b * P:(b + 1) * P], in_=pha[:, 0:P],
                        func=mybir.ActivationFunctionType.Relu,
                    )
                    nc.scalar.activation(
                        out=ht1[:, b * P:(b + 1) * P], in_=phb[:, 0:P],
                        func=mybir.ActivationFunctionType.Relu,
                    )
                # phase C: Y2 blocks -> ag2_in (deferred so the spmm matmul
                # stream above runs back-to-back on TensorE)
                for b in range(NBLK):
                    py2 = bank1(f"py2_{b}")
                    nc.tensor.matmul(
                        out=py2[:, 0:DEMB], lhsT=ht0[:, b * P:(b + 1) * P],
                        rhs=w2a[:], start=True, stop=False,
                    )
                    nc.tensor.matmul(
                        out=py2[:, 0:DEMB], lhsT=ht1[:, b * P:(b + 1) * P],
                        rhs=w2b[:], start=False, stop=True,
                    )
                    y2 = pb.tile([P, Y2W], BF16, tag="y2", bufs=2, name=f"y2_{b}")
                    nc.scalar.copy(out=y2[:, 0:DEMB], in_=py2[:, 0:DEMB])
                    nc.vector.memset(y2[:, DEMB:Y2W], 0)
                    nc.sync.dma_start(out=ag2_in[b * P:(b + 1) * P, :], in_=y2[:])

        if kphases >= 3:
            nc.gpsimd.collective_compute(
                "AllGather", mybir.AluOpType.bypass, replica_groups=rg,
                ins=[ag2_in.ap().opt()], outs=[ag2_out.ap().opt()],
            )

        wt2 = sb.tile([P, Y2W], BF16, tag="wt2", bufs=1)
        nc.sync.dma_start(out=wt2[:], in_=ag2_out[0:P, :])
        warm(W_AG2, "ag2", wt2, P, Y2W)

        if kphases >= 4:
            # ---- phase D: embT = (A @ Y2).T; c_loc = [embT; -sqh; -sql] ----
            c_loc = sb.tile([DEMB, R], BF16, tag="cloc", bufs=1)
            lhsT_all = sb.tile([DEMB + 2, R], BF16, tag="lhsT", bufs=1)
            bias_t = sb.tile([P, NBLK], F32, tag="bias", bufs=1)
            with tc.tile_pool(name="pd", bufs=1) as pd:
                negones = pd.tile([DEMB, 1], F32, tag="negones", bufs=1)
                nc.vector.memset(negones[:], -1.0)
                one1 = pd.tile([1, 1], F32, tag="one1", bufs=1)
                nc.vector.memset(one1[:], 1.0)
                sqrow = pd.tile([1, R], F32, tag="sqrow", bufs=1)
                sqh_b = pd.tile([1, R], BF16, tag="sqh_b", bufs=1)
                sql_b = pd.tile([1, R], BF16, tag="sql_b", bufs=1)
                g2s = []
                for b in range(NBLK):
                    k0 = b * b_ch
                    g2 = pd.tile([P, b_ch, Y2W], BF16, tag="g2", bufs=8,
                                 name=f"g2_{b}")
                    nc.gpsimd.dma_gather(
                        out_ap=g2[:],
                        in_ap=ag2_out[:, :],
                        idxs_ap=idx_t[:, k0 * 8:(k0 + b_ch) * 8],
                        num_idxs=b_ch * P,
                        num_idxs_reg=b_ch * P,
                        elem_size=Y2W,
                        single_packet=False,
                        queue_num=b % 4,
                    )
                    g2s.append(g2)
                for b in range(NBLK):
                    k0 = b * b_ch
                    g2 = g2s[b]
                    pe = bank1(f"pe_{b}")
                    for j in range(b_ch):
                        k = k0 + j
                        nc.tensor.matmul(
                            out=pe[0:DEMB, 0:P],
                            lhsT=g2[:, j, 0:DEMB],
                            rhs=s_all[:, k * P:(k + 1) * P],
                            start=(j == 0), stop=(j == b_ch - 1),
                        )
                    bsl = slice(b * P, (b + 1) * P)
                    nc.scalar.copy(out=c_loc[0:DEMB, bsl], in_=pe[0:DEMB, 0:P])
                # deferred per-block tails: lhsT rows, embT^2, -sq, bf16
                # hi/lo split, bias transpose
                for b in range(NBLK):
                    bsl = slice(b * P, (b + 1) * P)
                    sqt = pd.tile([DEMB, P], F32, tag="sqtmp", bufs=2,
                                  name=f"sqt_{b}")
                    nc.vector.tensor_scalar_mul(lhsT_all[0:DEMB, bsl],
                                                c_loc[0:DEMB, bsl], 2.0)
                    nc.scalar.square(out=sqt[:, 0:P],
                                     in_=c_loc[0:DEMB, bsl])
                    psq = bank1(f"psq_{b}")
                    nc.tensor.matmul(
                        out=psq[0:1, 0:P],
                        lhsT=negones[:],
                        rhs=sqt[:, 0:P],
                        start=True, stop=True,
                    )
                    nc.scalar.copy(out=sqrow[0:1, bsl], in_=psq[0:1, 0:P])
                    nc.scalar.copy(out=sqh_b[0:1, bsl], in_=psq[0:1, 0:P])
                    nc.vector.tensor_tensor(
                        out=sql_b[0:1, bsl],
                        in0=psq[0:1, 0:P],
                        in1=sqh_b[0:1, bsl],
                        op=mybir.AluOpType.subtract,
                    )
                    pbt = bank1(f"pbt_{b}")
                    nc.tensor.matmul(
                        out=pbt[:, 0:1], lhsT=sqrow[0:1, bsl],
                        rhs=one1[:], start=True, stop=True,
                    )
                    nc.vector.tensor_copy(bias_t[:, b:b + 1], pbt[:, 0:1])

                nc.sync.dma_start(out=ag3_in[0:DEMB, :], in_=c_loc[:])
                nc.sync.dma_start(out=ag3_in[DEMB:DEMB + 1, :], in_=sqh_b[:])
                nc.sync.dma_start(out=ag3_in[DEMB + 1:DEMB + 2, :], in_=sql_b[:])
                nc.gpsimd.collective_compute(
                    "AllGather", mybir.AluOpType.bypass, replica_groups=rg,
                    ins=[ag3_in.ap().opt()], outs=[ag3_out.ap().opt()],
                )
                nc.vector.memset(lhsT_all[DEMB:DEMB + 2, :], 1.0)
                wt3 = sb.tile([DEMB + 2, 512], BF16, tag="wt3", bufs=1)
                nc.sync.dma_start(out=wt3[:], in_=ag3_out[0:DEMB + 2, 0:512])
                warm(W_AG3, "ag3", wt3, DEMB + 2, 512)

        ps.release()
        if kphases >= 5:
            # ---- phase E: stripes of softmax(-dist) ------------------------
            NQ = 4          # chunks per stripe
            QW = N // NQ    # 2048 cols per chunk (2 PSUM banks x 2 bufs)
            with (
                tc.tile_pool(name="pef", bufs=1) as pef,
                tc.tile_pool(name="pse", bufs=1, space="PSUM") as pse,
            ):
                rhs_full = pef.tile([DEMB + 2, N], BF16, tag="rhs", bufs=1)
                for r in range(NC):
                    nc.sync.dma_start(
                        out=rhs_full[:, r * R:(r + 1) * R],
                        in_=ag3_out[r * (DEMB + 2):(r + 1) * (DEMB + 2), :],
                    )
                for s in range(NBLK):
                    stripe = pef.tile([P, N], BF16, tag="stripe", bufs=4,
                                      name=f"stripe_{s}")
                    sums = pef.tile([P, NQ], F32, tag="sums", bufs=2,
                                    name=f"sums_{s}")
                    for q in range(NQ):
                        pz = pse.tile([P, QW], F32, tag="pz", bufs=2,
                                      name=f"pz_{s}_{q}")
                        for jj in range(QW // 512):
                            c0 = q * QW + jj * 512
                            nc.tensor.matmul(
                                out=pz[:, jj * 512:(jj + 1) * 512],
                                lhsT=lhsT_all[:, s * P:(s + 1) * P],
                                rhs=rhs_full[:, c0:c0 + 512],
                                start=True, stop=True,
                            )
                        # row sums: split between ScalarE (fused accum_out)
                        # and VectorE (reduce) so neither engine owns all 4
                        if q < 2:
                            nc.scalar.activation(
                                out=stripe[:, q * QW:(q + 1) * QW],
                                in_=pz[:],
                                func=mybir.ActivationFunctionType.Exp,
                                bias=bias_t[:, s:s + 1],
                                scale=1.0,
                                accum_out=sums[:, q:q + 1],
                            )
                        else:
                            nc.scalar.activation(
                                out=stripe[:, q * QW:(q + 1) * QW],
                                in_=pz[:],
                                func=mybir.ActivationFunctionType.Exp,
                                bias=bias_t[:, s:s + 1],
                                scale=1.0,
                            )
                            nc.vector.tensor_reduce(
                                out=sums[:, q:q + 1],
                                in_=stripe[:, q * QW:(q + 1) * QW],
                                axis=mybir.AxisListType.X,
                                op=mybir.AluOpType.add,
                            )
                    stot = pef.tile([P, 1], F32, tag="stot", bufs=2,
                                    name=f"stot_{s}")
                    nc.vector.tensor_reduce(
                        out=stot[:], in_=sums[:], axis=mybir.AxisListType.X,
                        op=mybir.AluOpType.add,
                    )
                    rec = pef.tile([P, 1], F32, tag="rec", bufs=2,
                                   name=f"rec_{s}")
                    nc.vector.reciprocal(rec[:], stot[:])
                    # normalize + store per quarter so the output DMA starts
                    # as soon as the first quarter is scaled
                    for q in range(NQ):
                        qs = slice(q * QW, (q + 1) * QW)
                        nc.vector.tensor_scalar(
                            out=stripe[:, qs],
                            in0=stripe[:, qs],
                            scalar1=rec[:, 0:1],
                            scalar2=1e-10,
                            op0=mybir.AluOpType.mult,
                            op1=mybir.AluOpType.add,
                        )
                        nc.sync.dma_start(out=out[s * P:(s + 1) * P, qs],
                                          in_=stripe[:, qs])

    nc.finalize()
    return nc


def _block_unique(c, edge_src, edge_dst, edge_val):
    """Per 128-row block: unique src rows + (slot, dstrow, val) triples."""
    out = []
    sel = (edge_dst >= c * R) & (edge_dst < (c + 1) * R)
    src = edge_src[sel]
    val = edge_val[sel]
    loc = edge_dst[sel] - c * R
    blk = loc // P
    for b in range(NBLK):
        m = blk == b
        sb_, vb, rb = src[m], val[m], loc[m] % P
        uniq, slot = np.unique(sb_, return_inverse=True)
        out.append((uniq, slot, rb, vb))
    return out


def _prep_core(blocks, b_ch):
    """Pack per-block unique srcs into b_ch chunks; build multi-hot S."""
    c1 = NBLK * b_ch
    src_pad = np.zeros(c1 * P, np.int16)
    smat = np.zeros((c1 * P, P), np.float32)  # [slot, dstrow]
    for b, (uniq, slot, rb, vb) in enumerate(blocks):
        assert len(uniq) <= b_ch * P
        lo = b * b_ch * P
        src_pad[lo:lo + len(uniq)] = uniq.astype(np.int16)
        np.add.at(smat, (lo + slot, rb), vb)

    idx = np.tile(np.ascontiguousarray(src_pad.reshape(-1, 16).T), (8, 1))
    # device tile layout: [slot%128 partition, chunk*128 + dstrow free]
    smat_d = np.ascontiguousarray(
        smat.reshape(c1, P, P).transpose(1, 0, 2).reshape(P, c1 * P)
    ).astype(BFNP)
    return idx, smat_d


def kernel(X, W1, W2, edge_val, edge_src, edge_dst):
    global LAST_RESULTS
    X = np.asarray(X, np.float32)
    W1 = np.asarray(W1, np.float32)
    W2 = np.asarray(W2, np.float32)
    edge_val = np.asarray(edge_val, np.float32)
    edge_src = np.asarray(edge_src, np.int32)
    edge_dst = np.asarray(edge_dst, np.int32)

    # chunks per (core, block) from deduped src counts, uniform across cores
    per_core_blocks = [_block_unique(c, edge_src, edge_dst, edge_val)
                       for c in range(NC)]
    b_ch = max(int(np.ceil(len(u) / P)) for blocks in per_core_blocks
               for (u, _, _, _) in blocks)
    b_ch = max(b_ch, 1)

    if b_ch not in _GRAPH_CACHE:
        _GRAPH_CACHE[b_ch] = _build(b_ch)
    nc = _GRAPH_CACHE[b_ch]

    w1b = W1.astype(BFNP)
    w2b = W2.astype(BFNP)
    in_maps = []
    for c in range(NC):
        idx, smat_d = _prep_core(per_core_blocks[c], b_ch)
        in_maps.append({
            "xT": np.ascontiguousarray(X[c * R:(c + 1) * R].T).astype(BFNP),
            "w1": w1b,
            "w2": w2b,
            "idx": idx,
            "smat": smat_d,
        })

    trace = os.environ.get("KERNEL_TRACE", "0") == "1"
    res = run_bass_kernel_spmd(nc, in_maps, core_ids=list(range(NC)), trace=trace)
    LAST_RESULTS = res
    return np.concatenate(
        [res.results[c]["out"] for c in range(NC)], axis=0
    ).astype(np.float32)
